# revision 23
# baseline (speedup 1.0000x reference)
"""MoE fusion layer (per-sample top-2 routing) for 8 Trainium2 NeuronCores.

Strategy (data-parallel over B, one sample per core):
  - Routing (mean-pool -> 4 logits -> top-2 -> softmax gates) is computed on
    the host in float64 as part of input sharding: it determines which two
    expert weight matrices are shipped to each core.
  - Each core runs the identical SPMD Bass kernel on its own sample:
      mm1:  hT[H,L] = w1.T @ x      (fp16 operands, fp32 PSUM accumulation)
      act:  per-slot gated blend of {gelu, silu, mish}(hT + b1), selected by
            data (one-hot coefficients), decomposed onto one ACT table set:
              gelu(h) = 0.5*h*(1 + erf(h/sqrt2))
              silu(h) = h * s,                    s = sigmoid(h)
              mish(h) = h * (2/(1 + (1-s)^2) - 1)
            blend: aT = (C + A*erf + B*s + M*rec) * (h + b1)
      mm2:  y[L,D] += sum_k aT_k-slices @ w2_k  (PSUM-accumulated over both
            expert slots; gates folded into the blend coefficients)
      epilogue: y += x + sum_k g_k*b2_k (residual, folded into x on host),
            LayerNorm over D with E[y^2]-mean^2 variance and a bit-trick +
            Newton rsqrt on VectorE; ln_g/ln_b applied on host post-gather.
  - Gather: stack the 8 per-core [L, D] outputs into [B, L, D].

Everything data-dependent (expert choice, gate values, per-slot activation
selection) enters the kernel as *data*, so all 8 cores share one program.
"""

import numpy as np

B, L, D, E, TOP_K = 8, 1024, 768, 4, 2
H = 2 * D  # 1536
LN_EPS = 1e-5
P = 128
N_CORES = 8

ND, NH, NL = D // P, H // P, L // P  # 6, 12, 8
MM1_N = 512                 # mm1 moving free-dim chunk (PSUM bank limit)
MM2_CHUNKS = ((0, 512), (512, 768))  # mm2 free-dim chunks within one psum tile

# expert index -> activation kind (0=gelu(erf), 1=silu, 2=mish)
ACT_OF_EXPERT = (0, 1, 2, 0)

_CACHE = {}


def build_module(sim_acts=False):
    """Trace + compile the SPMD Bass/Tile module (one NeuronCore program).

    sim_acts=True substitutes Tanh for Erf (CoreSim lacks Erf); the test
    harness mirrors the substitution in its numpy model.
    """
    import concourse.bacc as bacc
    import concourse.mybir as mybir
    import concourse.tile as tile

    f32 = mybir.dt.float32
    f16 = mybir.dt.float16
    i32 = mybir.dt.int32
    AF = mybir.ActivationFunctionType
    ALU = mybir.AluOpType
    RSQ2 = 0.7071067811865476

    erf_func = AF.Tanh if sim_acts else AF.Erf

    nc = bacc.Bacc("TRN2", target_bir_lowering=False, debug=False)

    xT_d = nc.dram_tensor("xT16", [D, L], f16, kind="ExternalInput")
    xr_d = nc.dram_tensor("xres", [L, D], f32, kind="ExternalInput")
    w1_d = nc.dram_tensor("w1s", [TOP_K, D, H], f16, kind="ExternalInput")
    w2_d = nc.dram_tensor("w2s", [TOP_K, H, D], f16, kind="ExternalInput")
    b1_d = nc.dram_tensor("b1s", [P, 3 * TOP_K * NH], f32, kind="ExternalInput")
    sel_d = nc.dram_tensor("sel", [P, 8], f32, kind="ExternalInput")
    y_d = nc.dram_tensor("y", [L, D], f32, kind="ExternalOutput")

    with tile.TileContext(nc) as tc:
        from contextlib import ExitStack

        with ExitStack() as ctx:
            consts = ctx.enter_context(tc.tile_pool(name="consts", bufs=1))
            p_w1 = ctx.enter_context(tc.tile_pool(name="w1", bufs=TOP_K * ND))
            p_w2 = ctx.enter_context(tc.tile_pool(name="w2", bufs=TOP_K * NH))
            p_xT = ctx.enter_context(tc.tile_pool(name="xT", bufs=ND))
            p_xr = ctx.enter_context(tc.tile_pool(name="xr", bufs=4))
            p_aT = ctx.enter_context(tc.tile_pool(name="aT", bufs=TOP_K * NH))
            p_a16 = ctx.enter_context(tc.tile_pool(name="act16", bufs=2))
            p_bl = ctx.enter_context(tc.tile_pool(name="blend", bufs=2))
            p_ep = ctx.enter_context(tc.tile_pool(name="epil", bufs=2))
            p_st = ctx.enter_context(tc.tile_pool(name="stats", bufs=2))
            ps_h = ctx.enter_context(
                tc.tile_pool(name="ps_h", bufs=2, space="PSUM")
            )
            ps_y = ctx.enter_context(
                tc.tile_pool(name="ps_y", bufs=2, space="PSUM")
            )

            sel = consts.tile([P, 8], f32, name="sel")
            nc.sync.dma_start(sel[:], sel_d[:])
            b1sb = consts.tile([P, 3 * TOP_K * NH], f32, name="b1sb")
            nc.sync.dma_start(b1sb[:], b1_d[:])
            magicT = consts.tile([P, 1], i32, name="magicT")
            nc.vector.memset(magicT[:], 0x5F3759DF)

            # x^T (host-transposed), fp16 — mm1 moving operand; loaded first
            # so mm1 can start as early as possible
            xTt = []
            for dt_i in range(ND):
                t = p_xT.tile([P, L], f16, name=f"xT_{dt_i}", tag="xT")
                nc.sync.dma_start(t[:], xT_d[dt_i * P : (dt_i + 1) * P, :])
                xTt.append(t)
            # resident weights, fp16 (w1 before w2: mm1 needs it first)
            w1t = []
            for k in range(TOP_K):
                for dt_i in range(ND):
                    t = p_w1.tile([P, H], f16, name=f"w1_{k}_{dt_i}", tag="w1")
                    nc.sync.dma_start(
                        t[:], w1_d[k, dt_i * P : (dt_i + 1) * P, :]
                    )
                    w1t.append(t)
            w2t = []
            for k in range(TOP_K):
                for ht in range(NH):
                    t = p_w2.tile([P, D], f16, name=f"w2_{k}_{ht}", tag="w2")
                    nc.sync.dma_start(t[:], w2_d[k, ht * P : (ht + 1) * P, :])
                    w2t.append(t)

            # ---- mm1 + decomposed activations + gated blend ----
            aTt = []
            for k in range(TOP_K):
                for ht in range(NH):
                    psum_h = ps_h.tile([P, L], f32, name="psum_h", tag="ph")
                    for lc in range(L // MM1_N):
                        out_sl = psum_h[:, lc * MM1_N : (lc + 1) * MM1_N]
                        for dt_i in range(ND):
                            nc.tensor.matmul(
                                out_sl,
                                w1t[k * ND + dt_i][:, ht * P : (ht + 1) * P],
                                xTt[dt_i][:, lc * MM1_N : (lc + 1) * MM1_N],
                                start=(dt_i == 0),
                                stop=(dt_i == ND - 1),
                            )
                    ci = k * NH + ht
                    bias = b1sb[:, ci : ci + 1]
                    bias_s = b1sb[:, TOP_K * NH + ci : TOP_K * NH + ci + 1]
                    bias_n = b1sb[:, 2 * TOP_K * NH + ci : 2 * TOP_K * NH + ci + 1]
                    # hb = h + b1 ; e = erf(hb/sqrt2) ; m = sigmoid(-hb)
                    hb = p_a16.tile([P, L], f16, name="hb", tag="hb")
                    nc.scalar.activation(hb[:], psum_h[:], AF.Identity, bias=bias)
                    te = p_a16.tile([P, L], f16, name="te", tag="te")
                    nc.scalar.activation(
                        te[:], psum_h[:], erf_func, bias=bias_s, scale=RSQ2
                    )
                    tm = p_a16.tile([P, L], f16, name="tm", tag="tm")
                    nc.scalar.activation(
                        tm[:], psum_h[:], AF.Sigmoid, bias=bias_n, scale=-1.0
                    )
                    # mish factor rec = 1/(1 + m^2)  (m = 1 - sigmoid(hb))
                    q2 = p_a16.tile([P, L], f16, name="q2", tag="q2")
                    nc.scalar.activation(q2[:], tm[:], AF.Square)
                    den = p_bl.tile([P, L], f32, name="den", tag="den")
                    nc.vector.tensor_scalar_add(den[:], q2[:], 1.0)
                    rec = p_bl.tile([P, L], f32, name="rec", tag="rec")
                    nc.vector.reciprocal_approx_fast(out=rec[:], in_=den[:])
                    # inner = C' + A*e + B'*m + M*rec ; aT = inner * hb
                    t1 = p_bl.tile([P, L], f16, name="t1", tag="t1")
                    nc.vector.tensor_scalar(
                        t1[:], te[:], sel[:, 4 * k : 4 * k + 1],
                        sel[:, 4 * k + 3 : 4 * k + 4],
                        op0=ALU.mult, op1=ALU.add,
                    )
                    t2 = p_bl.tile([P, L], f16, name="t2", tag="t2")
                    nc.vector.scalar_tensor_tensor(
                        t2[:], tm[:], sel[:, 4 * k + 1 : 4 * k + 2], t1[:],
                        op0=ALU.mult, op1=ALU.add,
                    )
                    t3 = p_bl.tile([P, L], f16, name="t3", tag="t3")
                    nc.vector.scalar_tensor_tensor(
                        t3[:], rec[:], sel[:, 4 * k + 2 : 4 * k + 3], t2[:],
                        op0=ALU.mult, op1=ALU.add,
                    )
                    aT = p_aT.tile([P, L], f16, name=f"aT_{k}_{ht}", tag="aT")
                    nc.vector.tensor_mul(aT[:], t3[:], hb[:])
                    aTt.append(aT)

            # ---- mm2 + fused epilogue per 128-token tile ----
            n_acc = TOP_K * NH
            for Lt in range(NL):
                psum_y = ps_y.tile([P, 1024], f32, name="psum_y", tag="py")
                for m in range(n_acc):
                    lhsT = aTt[m][:, Lt * P : (Lt + 1) * P]
                    for c0, c1 in MM2_CHUNKS:
                        nc.tensor.matmul(
                            psum_y[:, c0:c1], lhsT, w2t[m][:, c0:c1],
                            start=(m == 0), stop=(m == n_acc - 1),
                        )

                xr = p_xr.tile([P, D], f32, name="xr", tag="xr")
                nc.sync.dma_start(xr[:], xr_d[Lt * P : (Lt + 1) * P, :])
                yA = p_ep.tile([P, D], f32, name="yA", tag="yA")
                msum = p_st.tile([P, 1], f32, name="msum", tag="msum")
                nc.vector.scalar_tensor_tensor(
                    yA[:], psum_y[:, 0:D], 1.0, xr[:],
                    op0=ALU.mult, op1=ALU.add, accum_out=msum[:],
                )
                ysq = p_ep.tile([P, D], f16, name="ysq", tag="ysq")
                vsum = p_st.tile([P, 1], f32, name="vsum", tag="vsum")
                nc.scalar.activation(ysq[:], yA[:], AF.Square, accum_out=vsum[:])
                # mu, var = E[y^2] - mu^2 ; rstd via bit-trick + Newton
                mu = p_st.tile([P, 1], f32, name="mu", tag="mu")
                nc.scalar.mul(mu[:], msum[:], 1.0 / D)
                musq = p_st.tile([P, 1], f32, name="musq", tag="musq")
                nc.vector.tensor_mul(musq[:], mu[:], mu[:])
                tv0 = p_st.tile([P, 1], f32, name="tv0", tag="tv0")
                nc.vector.tensor_scalar(
                    tv0[:], vsum[:], 1.0 / D, musq[:, 0:1],
                    op0=ALU.mult, op1=ALU.subtract,
                )
                tv = p_st.tile([P, 1], f32, name="tv", tag="tv")
                nc.vector.tensor_scalar_add(tv[:], tv0[:], LN_EPS)
                sh = p_st.tile([P, 1], i32, name="sh", tag="sh")
                nc.vector.tensor_scalar(
                    sh[:], tv[:].bitcast(i32), 1, None,
                    op0=ALU.arith_shift_right,
                )
                rstd = p_st.tile([P, 1], f32, name="rstd", tag="rstd")
                nc.vector.scalar_tensor_tensor(
                    rstd[:].bitcast(i32), sh[:], -1, magicT[:],
                    op0=ALU.mult, op1=ALU.add,
                )
                for _ in range(2):
                    nsq = p_st.tile([P, 1], f32, name="nsq", tag="nsq")
                    nc.vector.tensor_mul(nsq[:], rstd[:], rstd[:])
                    nb = p_st.tile([P, 1], f32, name="nb", tag="nb")
                    nc.vector.tensor_mul(nb[:], tv[:], nsq[:])
                    ncf = p_st.tile([P, 1], f32, name="ncf", tag="ncf")
                    nc.vector.tensor_scalar(
                        ncf[:], nb[:], -0.5, 1.5, op0=ALU.mult, op1=ALU.add
                    )
                    rstd2 = p_st.tile([P, 1], f32, name="rstd", tag="rstd")
                    nc.vector.tensor_mul(rstd2[:], rstd[:], ncf[:])
                    rstd = rstd2
                mrs = p_st.tile([P, 1], f32, name="mrs", tag="mrs")
                nc.vector.tensor_mul(mrs[:], mu[:], rstd[:])
                # out = y*rstd - mu*rstd   (ln_g/ln_b applied on host)
                o1 = p_ep.tile([P, D], f32, name="o1", tag="o1")
                nc.vector.tensor_scalar(
                    o1[:], yA[:], rstd[:, 0:1], mrs[:, 0:1],
                    op0=ALU.mult, op1=ALU.subtract,
                )
                nc.sync.dma_start(y_d[Lt * P : (Lt + 1) * P, :], o1[:])

    nc.compile()
    return nc


def route_host(x, router_w, router_b):
    """Per-sample top-2 routing in float64 (matches jax fp32 selection for
    any non-degenerate margin; margins on this data are >~1e-4 vs fp32
    noise ~1e-6)."""
    xf = x.astype(np.float64)
    pooled = xf.mean(axis=1)
    logits = pooled @ router_w.astype(np.float64).T + router_b.astype(np.float64)
    order = np.argsort(-logits, axis=1, kind="stable")
    top2 = order[:, :TOP_K]
    top_v = np.take_along_axis(logits, top2, axis=1)
    ex = np.exp(top_v - top_v.max(axis=1, keepdims=True))
    gates = ex / ex.sum(axis=1, keepdims=True)
    return top2, gates


def prepare_in_maps(x, router_w, router_b, w1, b1, w2, b2, ln_g, ln_b):
    top2, gates = route_host(x, router_w, router_b)
    w1_16 = w1.astype(np.float16)
    w2_16 = w2.astype(np.float16)

    in_maps = []
    for b_i in range(B):
        e0, e1 = int(top2[b_i, 0]), int(top2[b_i, 1])
        g0, g1 = float(gates[b_i, 0]), float(gates[b_i, 1])
        xb = x[b_i].astype(np.float32)

        # per-slot blend coefficients with m = sigmoid(-h) = 1 - s:
        #   inner = C' + A*erf + B'*m + M*rec
        #   gelu: 0.5g(1+erf)      -> A=0.5g, C'+=0.5g
        #   silu: g*s = g - g*m    -> B'=-g,  C'+=g
        #   mish: g*(2*rec-1)      -> M=2g,  C'-=g
        sel = np.zeros((P, 8), np.float32)
        for k, (e, g) in enumerate(((e0, g0), (e1, g1))):
            a_kind = ACT_OF_EXPERT[e]
            if a_kind == 0:
                sel[:, 4 * k + 0] = 0.5 * g
                sel[:, 4 * k + 3] = 0.5 * g
            elif a_kind == 1:
                sel[:, 4 * k + 1] = -g
                sel[:, 4 * k + 3] = g
            else:
                sel[:, 4 * k + 2] = 2.0 * g
                sel[:, 4 * k + 3] = -g

        b1s = np.zeros((P, 3 * TOP_K * NH), np.float32)
        for k, e in enumerate((e0, e1)):
            col = b1[e].astype(np.float64).reshape(NH, P).T
            b1s[:, k * NH : (k + 1) * NH] = col
            b1s[:, (TOP_K + k) * NH : (TOP_K + k + 1) * NH] = (
                col * 0.7071067811865476
            )
            b1s[:, (2 * TOP_K + k) * NH : (2 * TOP_K + k + 1) * NH] = -col

        resb = (
            g0 * b2[e0].astype(np.float64) + g1 * b2[e1].astype(np.float64)
        ).astype(np.float32)
        xres = (xb + resb[None, :]).astype(np.float32)

        in_maps.append(
            {
                "xT16": np.ascontiguousarray(xb.T).astype(np.float16),
                "xres": xres,
                "w1s": np.ascontiguousarray(w1_16[[e0, e1]]),
                "w2s": np.ascontiguousarray(w2_16[[e0, e1]]),
                "b1s": b1s,
                "sel": sel,
            }
        )
    return in_maps


def _get_module():
    if "nc" not in _CACHE:
        _CACHE["nc"] = build_module(sim_acts=False)
    return _CACHE["nc"]


def kernel(**inputs):
    x = np.asarray(inputs["x"])
    ln_g = np.asarray(inputs["ln_g"], np.float32)
    ln_b = np.asarray(inputs["ln_b"], np.float32)
    in_maps = prepare_in_maps(
        x,
        np.asarray(inputs["router_w"]),
        np.asarray(inputs["router_b"]),
        np.asarray(inputs["w1"]),
        np.asarray(inputs["b1"]),
        np.asarray(inputs["w2"]),
        np.asarray(inputs["b2"]),
        ln_g,
        ln_b,
    )
    from concourse.bass_utils import run_bass_kernel_spmd

    nc = _get_module()
    res = run_bass_kernel_spmd(nc, in_maps, core_ids=list(range(N_CORES)))
    out = np.stack(
        [res.results[i]["y"].astype(np.float32) for i in range(N_CORES)], axis=0
    )
    # ln_g/ln_b folded here (identity for this problem's setup, applied for
    # generality)
    out = out * ln_g[None, None, :] + ln_b[None, None, :]
    return out.astype(np.float32)


# revision 24
# speedup vs baseline: 1.0442x; 1.0442x over previous
"""MoE fusion layer (per-sample top-2 routing) for 8 Trainium2 NeuronCores.

Strategy (data-parallel over B, one sample per core):
  - Routing (mean-pool -> 4 logits -> top-2 -> softmax gates) is computed on
    the host in float64 as part of input sharding: it determines which two
    expert weight matrices are shipped to each core.
  - Each core runs the identical SPMD Bass kernel on its own sample:
      mm1:  hT[H,L] = w1.T @ x      (fp16 operands, fp32 PSUM accumulation)
      act:  per-slot gated blend of {gelu, silu, mish}(hT + b1), selected by
            data (one-hot coefficients), decomposed onto one ACT table set:
              gelu(h) = 0.5*h*(1 + erf(h/sqrt2))
              silu(h) = h * s,                    s = sigmoid(h)
              mish(h) = h * (2/(1 + (1-s)^2) - 1)
            blend: aT = (C + A*erf + B*s + M*rec) * (h + b1)
      mm2:  y[L,D] += sum_k aT_k-slices @ w2_k  (PSUM-accumulated over both
            expert slots; gates folded into the blend coefficients)
      epilogue: y += x + sum_k g_k*b2_k (residual, folded into x on host),
            LayerNorm over D with E[y^2]-mean^2 variance and a bit-trick +
            Newton rsqrt on VectorE; ln_g/ln_b applied on host post-gather.
  - Gather: stack the 8 per-core [L, D] outputs into [B, L, D].

Everything data-dependent (expert choice, gate values, per-slot activation
selection) enters the kernel as *data*, so all 8 cores share one program.
"""

import numpy as np

B, L, D, E, TOP_K = 8, 1024, 768, 4, 2
H = 2 * D  # 1536
LN_EPS = 1e-5
P = 128
N_CORES = 8

ND, NH, NL = D // P, H // P, L // P  # 6, 12, 8
MM1_N = 512                 # mm1 moving free-dim chunk (PSUM bank limit)
MM2_CHUNKS = ((0, 512), (512, 768))  # mm2 free-dim chunks within one psum tile

# expert index -> activation kind (0=gelu(erf), 1=silu, 2=mish)
ACT_OF_EXPERT = (0, 1, 2, 0)

_CACHE = {}


def build_module(sim_acts=False):
    """Trace + compile the SPMD Bass/Tile module (one NeuronCore program).

    sim_acts=True substitutes Tanh for Erf (CoreSim lacks Erf); the test
    harness mirrors the substitution in its numpy model.
    """
    import concourse.bacc as bacc
    import concourse.mybir as mybir
    import concourse.tile as tile

    f32 = mybir.dt.float32
    f16 = mybir.dt.float16
    i32 = mybir.dt.int32
    AF = mybir.ActivationFunctionType
    ALU = mybir.AluOpType
    RSQ2 = 0.7071067811865476

    erf_func = AF.Tanh if sim_acts else AF.Erf

    nc = bacc.Bacc("TRN2", target_bir_lowering=False, debug=False)

    xT_d = nc.dram_tensor("xT16", [D, L], f16, kind="ExternalInput")
    xr_d = nc.dram_tensor("xres", [L, D], f32, kind="ExternalInput")
    w1_d = nc.dram_tensor("w1s", [TOP_K, D, H], f16, kind="ExternalInput")
    w2_d = nc.dram_tensor("w2s", [TOP_K, H, D], f16, kind="ExternalInput")
    b1_d = nc.dram_tensor("b1s", [P, 3 * TOP_K * NH], f32, kind="ExternalInput")
    sel_d = nc.dram_tensor("sel", [P, 8], f32, kind="ExternalInput")
    y_d = nc.dram_tensor("y", [L, D], f32, kind="ExternalOutput")

    with tile.TileContext(nc) as tc:
        from contextlib import ExitStack

        with ExitStack() as ctx:
            consts = ctx.enter_context(tc.tile_pool(name="consts", bufs=1))
            p_w1 = ctx.enter_context(tc.tile_pool(name="w1", bufs=TOP_K * ND))
            p_w2 = ctx.enter_context(tc.tile_pool(name="w2", bufs=TOP_K * NH))
            p_xT = ctx.enter_context(tc.tile_pool(name="xT", bufs=ND))
            p_xr = ctx.enter_context(tc.tile_pool(name="xr", bufs=2))
            p_aT = ctx.enter_context(tc.tile_pool(name="aT", bufs=TOP_K * NH))
            p_a16 = ctx.enter_context(tc.tile_pool(name="act16", bufs=3))
            p_bl = ctx.enter_context(tc.tile_pool(name="blend", bufs=2))
            p_ep = ctx.enter_context(tc.tile_pool(name="epil", bufs=2))
            p_st = ctx.enter_context(tc.tile_pool(name="stats", bufs=2))
            ps_h = ctx.enter_context(
                tc.tile_pool(name="ps_h", bufs=2, space="PSUM")
            )
            ps_y = ctx.enter_context(
                tc.tile_pool(name="ps_y", bufs=2, space="PSUM")
            )

            sel = consts.tile([P, 8], f32, name="sel")
            nc.sync.dma_start(sel[:], sel_d[:])
            b1sb = consts.tile([P, 3 * TOP_K * NH], f32, name="b1sb")
            nc.sync.dma_start(b1sb[:], b1_d[:])
            magicT = consts.tile([P, 1], i32, name="magicT")
            nc.vector.memset(magicT[:], 0x5F3759DF)

            # x^T (host-transposed), fp16 — mm1 moving operand; loaded first
            # so mm1 can start as early as possible
            xTt = []
            for dt_i in range(ND):
                t = p_xT.tile([P, L], f16, name=f"xT_{dt_i}", tag="xT")
                nc.sync.dma_start(t[:], xT_d[dt_i * P : (dt_i + 1) * P, :])
                xTt.append(t)
            # resident weights, fp16 (w1 before w2: mm1 needs it first)
            w1t = []
            for k in range(TOP_K):
                for dt_i in range(ND):
                    t = p_w1.tile([P, H], f16, name=f"w1_{k}_{dt_i}", tag="w1")
                    nc.sync.dma_start(
                        t[:], w1_d[k, dt_i * P : (dt_i + 1) * P, :]
                    )
                    w1t.append(t)
            w2t = []
            for k in range(TOP_K):
                for ht in range(NH):
                    t = p_w2.tile([P, D], f16, name=f"w2_{k}_{ht}", tag="w2")
                    nc.sync.dma_start(t[:], w2_d[k, ht * P : (ht + 1) * P, :])
                    w2t.append(t)

            # ---- mm1 + decomposed activations + gated blend ----
            aTt = []
            for k in range(TOP_K):
                for ht in range(NH):
                    psum_h = ps_h.tile([P, L], f32, name="psum_h", tag="ph")
                    for lc in range(L // MM1_N):
                        out_sl = psum_h[:, lc * MM1_N : (lc + 1) * MM1_N]
                        for dt_i in range(ND):
                            nc.tensor.matmul(
                                out_sl,
                                w1t[k * ND + dt_i][:, ht * P : (ht + 1) * P],
                                xTt[dt_i][:, lc * MM1_N : (lc + 1) * MM1_N],
                                start=(dt_i == 0),
                                stop=(dt_i == ND - 1),
                            )
                    ci = k * NH + ht
                    bias = b1sb[:, ci : ci + 1]
                    bias_s = b1sb[:, TOP_K * NH + ci : TOP_K * NH + ci + 1]
                    bias_n = b1sb[:, 2 * TOP_K * NH + ci : 2 * TOP_K * NH + ci + 1]
                    # hb = h + b1 ; e = erf(hb/sqrt2) ; m = sigmoid(-hb)
                    hb = p_a16.tile([P, L], f16, name="hb", tag="hb")
                    nc.scalar.activation(hb[:], psum_h[:], AF.Identity, bias=bias)
                    te = p_a16.tile([P, L], f16, name="te", tag="te")
                    nc.scalar.activation(
                        te[:], psum_h[:], erf_func, bias=bias_s, scale=RSQ2
                    )
                    tm = p_a16.tile([P, L], f16, name="tm", tag="tm")
                    nc.scalar.activation(
                        tm[:], psum_h[:], AF.Sigmoid, bias=bias_n, scale=-1.0
                    )
                    # mish factor rec = 1/(1 + m^2)  (m = 1 - sigmoid(hb))
                    q2 = p_a16.tile([P, L], f16, name="q2", tag="q2")
                    nc.scalar.activation(q2[:], tm[:], AF.Square)
                    den = p_bl.tile([P, L], f32, name="den", tag="den")
                    nc.vector.tensor_scalar_add(den[:], q2[:], 1.0)
                    rec = p_bl.tile([P, L], f32, name="rec", tag="rec")
                    nc.vector.reciprocal_approx_fast(out=rec[:], in_=den[:])
                    # inner = C' + A*e + B'*m + M*rec ; aT = inner * hb
                    t1 = p_bl.tile([P, L], f16, name="t1", tag="t1")
                    nc.vector.tensor_scalar(
                        t1[:], te[:], sel[:, 4 * k : 4 * k + 1],
                        sel[:, 4 * k + 3 : 4 * k + 4],
                        op0=ALU.mult, op1=ALU.add,
                    )
                    t2 = p_bl.tile([P, L], f16, name="t2", tag="t2")
                    nc.vector.scalar_tensor_tensor(
                        t2[:], tm[:], sel[:, 4 * k + 1 : 4 * k + 2], t1[:],
                        op0=ALU.mult, op1=ALU.add,
                    )
                    t3 = p_bl.tile([P, L], f16, name="t3", tag="t3")
                    nc.vector.scalar_tensor_tensor(
                        t3[:], rec[:], sel[:, 4 * k + 2 : 4 * k + 3], t2[:],
                        op0=ALU.mult, op1=ALU.add,
                    )
                    aT = p_aT.tile([P, L], f16, name=f"aT_{k}_{ht}", tag="aT")
                    nc.vector.tensor_mul(aT[:], t3[:], hb[:])
                    aTt.append(aT)

            # ---- mm2 + fused epilogue per 128-token tile ----
            n_acc = TOP_K * NH
            for Lt in range(NL):
                psum_y = ps_y.tile([P, 1024], f32, name="psum_y", tag="py")
                for m in range(n_acc):
                    lhsT = aTt[m][:, Lt * P : (Lt + 1) * P]
                    for c0, c1 in MM2_CHUNKS:
                        nc.tensor.matmul(
                            psum_y[:, c0:c1], lhsT, w2t[m][:, c0:c1],
                            start=(m == 0), stop=(m == n_acc - 1),
                        )

                xr = p_xr.tile([P, D], f32, name="xr", tag="xr")
                nc.sync.dma_start(xr[:], xr_d[Lt * P : (Lt + 1) * P, :])
                yA = p_ep.tile([P, D], f32, name="yA", tag="yA")
                msum = p_st.tile([P, 1], f32, name="msum", tag="msum")
                nc.vector.scalar_tensor_tensor(
                    yA[:], psum_y[:, 0:D], 1.0, xr[:],
                    op0=ALU.mult, op1=ALU.add, accum_out=msum[:],
                )
                ysq = p_ep.tile([P, D], f16, name="ysq", tag="ysq")
                vsum = p_st.tile([P, 1], f32, name="vsum", tag="vsum")
                nc.scalar.activation(ysq[:], yA[:], AF.Square, accum_out=vsum[:])
                # mu, var = E[y^2] - mu^2 ; rstd via bit-trick + Newton
                mu = p_st.tile([P, 1], f32, name="mu", tag="mu")
                nc.scalar.mul(mu[:], msum[:], 1.0 / D)
                musq = p_st.tile([P, 1], f32, name="musq", tag="musq")
                nc.vector.tensor_mul(musq[:], mu[:], mu[:])
                tv0 = p_st.tile([P, 1], f32, name="tv0", tag="tv0")
                nc.vector.tensor_scalar(
                    tv0[:], vsum[:], 1.0 / D, musq[:, 0:1],
                    op0=ALU.mult, op1=ALU.subtract,
                )
                tv = p_st.tile([P, 1], f32, name="tv", tag="tv")
                nc.vector.tensor_scalar_add(tv[:], tv0[:], LN_EPS)
                sh = p_st.tile([P, 1], i32, name="sh", tag="sh")
                nc.vector.tensor_scalar(
                    sh[:], tv[:].bitcast(i32), 1, None,
                    op0=ALU.arith_shift_right,
                )
                rstd = p_st.tile([P, 1], f32, name="rstd", tag="rstd")
                nc.vector.scalar_tensor_tensor(
                    rstd[:].bitcast(i32), sh[:], -1, magicT[:],
                    op0=ALU.mult, op1=ALU.add,
                )
                for _ in range(2):
                    nsq = p_st.tile([P, 1], f32, name="nsq", tag="nsq")
                    nc.vector.tensor_mul(nsq[:], rstd[:], rstd[:])
                    nb = p_st.tile([P, 1], f32, name="nb", tag="nb")
                    nc.vector.tensor_mul(nb[:], tv[:], nsq[:])
                    ncf = p_st.tile([P, 1], f32, name="ncf", tag="ncf")
                    nc.vector.tensor_scalar(
                        ncf[:], nb[:], -0.5, 1.5, op0=ALU.mult, op1=ALU.add
                    )
                    rstd2 = p_st.tile([P, 1], f32, name="rstd", tag="rstd")
                    nc.vector.tensor_mul(rstd2[:], rstd[:], ncf[:])
                    rstd = rstd2
                mrs = p_st.tile([P, 1], f32, name="mrs", tag="mrs")
                nc.vector.tensor_mul(mrs[:], mu[:], rstd[:])
                # out = y*rstd - mu*rstd   (ln_g/ln_b applied on host)
                o1 = p_ep.tile([P, D], f32, name="o1", tag="o1")
                nc.vector.tensor_scalar(
                    o1[:], yA[:], rstd[:, 0:1], mrs[:, 0:1],
                    op0=ALU.mult, op1=ALU.subtract,
                )
                nc.sync.dma_start(y_d[Lt * P : (Lt + 1) * P, :], o1[:])

    nc.compile()
    return nc


def route_host(x, router_w, router_b):
    """Per-sample top-2 routing in float64 (matches jax fp32 selection for
    any non-degenerate margin; margins on this data are >~1e-4 vs fp32
    noise ~1e-6)."""
    xf = x.astype(np.float64)
    pooled = xf.mean(axis=1)
    logits = pooled @ router_w.astype(np.float64).T + router_b.astype(np.float64)
    order = np.argsort(-logits, axis=1, kind="stable")
    top2 = order[:, :TOP_K]
    top_v = np.take_along_axis(logits, top2, axis=1)
    ex = np.exp(top_v - top_v.max(axis=1, keepdims=True))
    gates = ex / ex.sum(axis=1, keepdims=True)
    return top2, gates


def prepare_in_maps(x, router_w, router_b, w1, b1, w2, b2, ln_g, ln_b):
    top2, gates = route_host(x, router_w, router_b)
    w1_16 = w1.astype(np.float16)
    w2_16 = w2.astype(np.float16)

    in_maps = []
    for b_i in range(B):
        e0, e1 = int(top2[b_i, 0]), int(top2[b_i, 1])
        g0, g1 = float(gates[b_i, 0]), float(gates[b_i, 1])
        xb = x[b_i].astype(np.float32)

        # per-slot blend coefficients with m = sigmoid(-h) = 1 - s:
        #   inner = C' + A*erf + B'*m + M*rec
        #   gelu: 0.5g(1+erf)      -> A=0.5g, C'+=0.5g
        #   silu: g*s = g - g*m    -> B'=-g,  C'+=g
        #   mish: g*(2*rec-1)      -> M=2g,  C'-=g
        sel = np.zeros((P, 8), np.float32)
        for k, (e, g) in enumerate(((e0, g0), (e1, g1))):
            a_kind = ACT_OF_EXPERT[e]
            if a_kind == 0:
                sel[:, 4 * k + 0] = 0.5 * g
                sel[:, 4 * k + 3] = 0.5 * g
            elif a_kind == 1:
                sel[:, 4 * k + 1] = -g
                sel[:, 4 * k + 3] = g
            else:
                sel[:, 4 * k + 2] = 2.0 * g
                sel[:, 4 * k + 3] = -g

        b1s = np.zeros((P, 3 * TOP_K * NH), np.float32)
        for k, e in enumerate((e0, e1)):
            col = b1[e].astype(np.float64).reshape(NH, P).T
            b1s[:, k * NH : (k + 1) * NH] = col
            b1s[:, (TOP_K + k) * NH : (TOP_K + k + 1) * NH] = (
                col * 0.7071067811865476
            )
            b1s[:, (2 * TOP_K + k) * NH : (2 * TOP_K + k + 1) * NH] = -col

        resb = (
            g0 * b2[e0].astype(np.float64) + g1 * b2[e1].astype(np.float64)
        ).astype(np.float32)
        xres = (xb + resb[None, :]).astype(np.float32)

        in_maps.append(
            {
                "xT16": np.ascontiguousarray(xb.T).astype(np.float16),
                "xres": xres,
                "w1s": np.ascontiguousarray(w1_16[[e0, e1]]),
                "w2s": np.ascontiguousarray(w2_16[[e0, e1]]),
                "b1s": b1s,
                "sel": sel,
            }
        )
    return in_maps


def _get_module():
    if "nc" not in _CACHE:
        _CACHE["nc"] = build_module(sim_acts=False)
    return _CACHE["nc"]


def kernel(**inputs):
    x = np.asarray(inputs["x"])
    ln_g = np.asarray(inputs["ln_g"], np.float32)
    ln_b = np.asarray(inputs["ln_b"], np.float32)
    in_maps = prepare_in_maps(
        x,
        np.asarray(inputs["router_w"]),
        np.asarray(inputs["router_b"]),
        np.asarray(inputs["w1"]),
        np.asarray(inputs["b1"]),
        np.asarray(inputs["w2"]),
        np.asarray(inputs["b2"]),
        ln_g,
        ln_b,
    )
    from concourse.bass_utils import run_bass_kernel_spmd

    nc = _get_module()
    res = run_bass_kernel_spmd(nc, in_maps, core_ids=list(range(N_CORES)))
    out = np.stack(
        [res.results[i]["y"].astype(np.float32) for i in range(N_CORES)], axis=0
    )
    # ln_g/ln_b folded here (identity for this problem's setup, applied for
    # generality)
    out = out * ln_g[None, None, :] + ln_b[None, None, :]
    return out.astype(np.float32)
